# revision 1
# baseline (speedup 1.0000x reference)
"""Trainium2 Bass kernel for nn_Encoder_23124103922122 (segment_reduce), v2.

Math (per rank r of 6, labels lab_r[0..4095] in [0,256)):
    seg_r[b, g]  = sum_{i: lab_r[i]==g} F[b, i]          (segment sum)
    out[b, j, r] = seg_r[b, lab_r[j]]                     (gather back)
    out[b, j, 6] = F[b, j]                                (identity channel)

v2 design (all compute in bf16; ~1.3e-3 rel err from bf16 rounding):
  - Host passes F pre-transposed+cast: f_t[p, t*128+b] = F[b, t*128+p] bf16.
    No on-device transposes or casts of F at all; F f32 is never loaded.
  - Stage 1 runs mask-as-STATIONARY: psum_segT[g_half, b] += m1[i, g_half].T
    @ f_t[i, b], producing seg_T directly (no seg transposes). Masks built
    on DVE via tensor_scalar(is_equal) in bf16 -> 4x_2p mode (4 elem/cyc);
    ~22% of masks offloaded to the (otherwise idle) GPSIMD/Pool engine.
  - Stage 2: m2[g, j] = (lab[j]==g) built from a partition-broadcast label
    table (bf16, 4x mode) in 1024-wide strips; out chunks are per-rank
    matmuls into PSUM, interleaved into [b, j, 7] SBUF staging by strided
    copies split across ACT and DVE, then streamed to HBM in j-chunks.
    Identity channel comes from a PE matmul of f_t against the identity.
  - PE is kept continuously busy from ~0.2us (identity warm-up matmuls) so
    it is fully ramped when stage 1 starts; stage 1 is PE/DVE-co-paced and
    finishes ~23us, after which the 40.8us output stream is the wall.

Sharding: data-parallel over batch B=1024 -> 8 cores x 128 rows. Labels &
tables replicated. No cross-device communication.

Note: walrus in this container accepts at most ONE sync-wait per instruction
(two on EventSemaphore); _legalize_waits() post-processes the Tile-scheduled
program to satisfy that.
"""

import sys

if "/opt/trn_rl_repo" not in sys.path:
    sys.path.insert(0, "/opt/trn_rl_repo")

from contextlib import ExitStack

import ml_dtypes
import numpy as np

import concourse.bass as bass
import concourse.mybir as mybir
import concourse.tile as tile
from concourse.bass import ts
from concourse.bass_utils import run_bass_kernel_spmd

B, N, R, G = 1024, 4096, 6, 256
NCORES = 8
BL = B // NCORES  # 128 batch rows per core
P = 128
NT = N // P  # 32 genus tiles
F32 = mybir.dt.float32
BF16 = mybir.dt.bfloat16
U8 = mybir.dt.uint8

# j-chunk widths for the output stream: small head chunks so the first DMA
# fires early, small tail chunks for a short final drain. Chunk boundaries
# align with the 1024-wide stage-2 mask strips.
WIDTHS = [128, 128, 256, 512, 512, 512, 512, 512, 512, 256, 256]
assert sum(WIDTHS) == N
STRIP = 512  # stage-2 mask strip width
# after finishing chunk c, build these stage-2 strips
STRIP_AFTER_CHUNK = {1: (1,), 3: (2,), 4: (3,), 5: (4, 5), 6: (6, 7)}

# Within each rank, stage-1 mask tiles are split across engines so the mask
# stream keeps pace with PE (~3.4us/rank): DVE builds 23 (127ns each), Pool 6
# (451ns), ACT 4 (two-op abs/relu trick, ~800ns).
POOL_T = {7, 15, 23, 29, 31}
ACT_T = {3, 13, 27}

_cache: dict = {}

# Engine -> prefix of the semaphore names its compute instructions increment.
# Pool (GPSIMD) is excluded: its 8 DSP cores do not guarantee in-order
# completion, so Pool-on-Pool waits cannot be dropped as redundant.
_ENGINE_SEM_PREFIX = {
    mybir.EngineType.PE: "PE",
    mybir.EngineType.DVE: "DVE",
    mybir.EngineType.Activation: "Activation",
    mybir.EngineType.SP: "SP",
}


def _legalize_waits(nc):
    """Walrus only accepts 1 sync-wait per instruction (2 on EventSemaphore),
    but the Tile scheduler can emit more. Post-pass:
      1. drop waits on the instruction's own engine semaphore that are already
         satisfied by same-engine program order (compute completion is in-order
         and sem targets are absolute), and
      2. hoist remaining excess waits onto EventSemaphore carrier instructions
         inserted just before the instruction on the same engine.
    """
    ev_id = 0
    for f in nc.m.functions:
        for blk in f.blocks:
            insts = blk.instructions
            sem_incs: dict = {}  # (engine, sem_name) -> cumulative inc in stream
            new_insts = []
            for inst in insts:
                si = inst.sync_info
                if si is not None and si.on_wait:
                    cap = 2 if isinstance(inst, mybir.InstEventSemaphore) else 1
                    eng = inst.engine
                    pfx = _ENGINE_SEM_PREFIX.get(eng)
                    kept = []
                    for w in si.on_wait:
                        sem_eng = w.ant_name.rsplit("_", 1)[0]
                        if (
                            pfx is not None
                            and sem_eng == pfx
                            and w.wait_mode == "sem-ge-imm"
                            and sem_incs.get((eng, w.ant_name), 0) >= w.wait_value
                        ):
                            continue  # satisfied by same-engine execution order
                        kept.append(w)
                    while len(kept) > cap:
                        ncarry = min(2, len(kept) - cap + 1)
                        carry, kept = kept[:ncarry], kept[ncarry:]
                        ev = mybir.InstEventSemaphore(
                            name=f"EVW-{ev_id}", ins=[], outs=[]
                        )
                        ev_id += 1
                        ev.engine = eng
                        ev.sync_info = mybir.SyncInfo(on_wait=carry, on_update=[])
                        new_insts.append(ev)
                    inst.sync_info = mybir.SyncInfo(
                        on_wait=kept, on_update=si.on_update
                    )
                si = inst.sync_info
                if si is not None:
                    for u in si.on_update:
                        if u.update_mode == "sem-inc":
                            key = (inst.engine, u.ant_name)
                            sem_incs[key] = sem_incs.get(key, 0) + u.update_value
                new_insts.append(inst)
            if len(new_insts) != len(insts):
                insts[:] = new_insts


def _build_nc():
    nc = bass.Bass("TRN2", debug=False, num_devices=NCORES)

    # f_t[p, t*128 + b] = F[b, t*128 + p]  (bf16 transposed F tiles)
    f_t_in = nc.dram_tensor("f_t_in", [P, N], BF16, kind="ExternalInput").ap()
    # tabs_bf[p, 0:G] = iota_g; tabs_bf[p, G:G+P] = identity (bf16)
    tabs_bf = nc.dram_tensor("tabs_bf", [P, G + P], BF16, kind="ExternalInput").ap()
    # tabs_f32[p, 0:2] = (p, p+128); tabs_f32[p, 2 + r*NT + t] = labels[r, t*128+p]
    tabs_f32 = nc.dram_tensor(
        "tabs_f32", [P, 2 + R * NT], F32, kind="ExternalInput"
    ).ap()
    # lab_bf[r, j] = labels[r, j] (uint8, partition-broadcast source; u8 keeps
    # the 3MB broadcast off the critical path at the cost of 2x instead of 4x
    # DVE mode for the stage-2 mask builds)
    lab_bf = nc.dram_tensor("lab_bf", [R, N], U8, kind="ExternalInput").ap()
    out = nc.dram_tensor("out", [BL, N, R + 1], F32, kind="ExternalOutput").ap()

    with ExitStack() as ctx:
        tc = ctx.enter_context(tile.TileContext(nc))

        const = ctx.enter_context(tc.tile_pool(name="const", bufs=1))
        m1p = ctx.enter_context(tc.tile_pool(name="m1p", bufs=56))
        m1pp = ctx.enter_context(tc.tile_pool(name="m1pp", bufs=24))
        m1pa = ctx.enter_context(tc.tile_pool(name="m1pa", bufs=16))
        segp = ctx.enter_context(tc.tile_pool(name="segp", bufs=1))
        m2p = ctx.enter_context(tc.tile_pool(name="m2p", bufs=4))
        outp = ctx.enter_context(tc.tile_pool(name="outp", bufs=4))

        # ---- input DMAs, all on the SP queue so they issue strictly in
        # order: tables, then f_t (compute-critical), then the 6 per-rank
        # label broadcasts (stage 2 needs rank r's labels only once stage 1
        # is nearly done, and the broadcasts complete rank-by-rank). ----
        tf32_sb = const.tile([P, 2 + R * NT], F32)
        nc.sync.dma_start(tf32_sb[:], tabs_f32)
        tbf_sb = const.tile([P, G + P], BF16)
        nc.scalar.dma_start(tbf_sb[:], tabs_bf)
        f_t = const.tile([P, N], BF16)
        NPC = 4
        f_dmas = [
            nc.sync.dma_start(
                f_t[:, q * (N // NPC) : (q + 1) * (N // NPC)],
                f_t_in[:, q * (N // NPC) : (q + 1) * (N // NPC)],
            )
            for q in range(NPC)
        ]
        from concourse.tile import add_dep_helper

        lab_bc = const.tile([P, R, N], U8)
        for r in range(R):
            nc.sync.dma_start(
                lab_bc[:, r : r + 1, :], lab_bf[r : r + 1, :].partition_broadcast(P)
            )

        iota_p_sb = tf32_sb[:, 0:2]
        labT_sb = tf32_sb[:, 2:]
        iota_g_sb = tbf_sb[:, 0:G]
        ident_sb = tbf_sb[:, G:]

        # ---- prewarm: absorb const-DMA semaphores into the DVE/Pool clocks
        # (hot-loop ops may carry at most one sync wait), and keep PE busy on
        # identity matmuls until f_t arrives so its p-state ramps. ----
        warm = const.tile([P, 4], BF16)
        nc.vector.tensor_copy(warm[:, 0:1], tf32_sb[:, 0:1])
        nc.vector.tensor_copy(warm[:, 1:2], tbf_sb[:, 0:1])
        nc.gpsimd.tensor_copy(warm[:, 2:3], tf32_sb[:, 1:2])
        nc.gpsimd.tensor_copy(warm[:, 3:4], tbf_sb[:, 1:2])
        ps_warm_ctx = tc.tile_pool(name="ps_warm", bufs=1, space="PSUM")
        ps_warm = ps_warm_ctx.__enter__()
        wps = ps_warm.tile([P, P], F32)
        # p-state warm-up on an UNINITIALIZED tile: no input dependency, so PE
        # is busy from ~0.4us and fully ramped before the first stage-1 matmul
        # (values are zero and never read)
        wjunk = const.tile([P, P], BF16)
        nc.vector.memset(wjunk[:], 0.0)
        for _ in range(24):
            nc.tensor.matmul(wps[:], wjunk[:], wjunk[:], start=True, stop=True)

        # ---- stage 1 + stage-2 mask strips ----
        seg_t = []
        # strips[s][(r, gh)] = [128, STRIP] one-hot m2 tiles for j in
        # [s*STRIP, (s+1)*STRIP)
        strips = [dict() for _ in range(N // STRIP)]

        last_strip_op = [None] * (N // STRIP)

        def emit_strip_ops(s, ranks, order_dep=None, lo=0, hi=STRIP):
            j0 = s * STRIP
            for r in ranks:
                for gh in range(2):
                    if (r, gh) in strips[s]:
                        m2 = strips[s][(r, gh)]
                    else:
                        m2 = m2p.tile(
                            [P, STRIP], BF16, tag=f"m2_{r}_{gh}",
                            name=f"m2_{s}_{r}_{gh}",
                        )
                        strips[s][(r, gh)] = m2
                    on_pool = s == 1 or (s >= 2 and gh == 1 and r % 2 == 1)
                    eng = nc.gpsimd if on_pool else nc.vector
                    op = eng.tensor_scalar(
                        m2[:, lo:hi],
                        lab_bc[:, r, j0 + lo : j0 + hi],
                        iota_p_sb[:, gh : gh + 1],
                        None,
                        op0=mybir.AluOpType.is_equal,
                    )
                    if order_dep is not None and not on_pool:
                        add_dep_helper(op.ins, order_dep.ins, reason="strip order")
                    if not on_pool:
                        last_strip_op[s] = op

        with tc.tile_pool(name="ps_seg", bufs=1, space="PSUM") as ps_seg:
            seg_ps = {}
            for r in range(R):
                t_ = ps_seg.tile([P, 2, P], F32, tag=f"segps{r}", name=f"segps{r}")
                for gh in range(2):
                    seg_ps[(r, gh)] = t_[:, gh, :]

            last_dve_mask = [None] * R

            def emit_masks(r):
                masks = []
                pool_t = POOL_T | {3, 19} if r == R - 1 else POOL_T
                for t in range(NT):
                    col = r * NT + t
                    if t in pool_t:
                        mt = m1pp.tile([P, G], BF16, tag="m1p", name=f"m1p{r}_{t}")
                        nc.gpsimd.tensor_scalar(
                            mt[:],
                            iota_g_sb[:],
                            labT_sb[:, col : col + 1],
                            None,
                            op0=mybir.AluOpType.is_equal,
                        )
                    elif t in ACT_T and r != R - 1:
                        # mask = relu(1 - |iota - lab|), exact for integers
                        mt = m1pa.tile([P, G], BF16, tag="m1a", name=f"m1a{r}_{t}")
                        tabs_ = m1pa.tile([P, G], BF16, tag="mabs", name=f"ma{r}_{t}")
                        nc.scalar.activation(
                            tabs_[:],
                            iota_g_sb[:],
                            mybir.ActivationFunctionType.Abs,
                            bias=labT_sb[:, col : col + 1],
                            scale=-1.0,
                        )
                        nc.scalar.activation(
                            mt[:],
                            tabs_[:],
                            mybir.ActivationFunctionType.Relu,
                            bias=1.0,
                            scale=-1.0,
                        )
                    else:
                        mt = m1p.tile([P, G], BF16, tag="m1", name=f"m1_{r}_{t}")
                        op = nc.vector.tensor_scalar(
                            mt[:],
                            iota_g_sb[:],
                            labT_sb[:, col : col + 1],
                            None,
                            op0=mybir.AluOpType.is_equal,
                        )
                        last_dve_mask[r] = op
                    masks.append(mt)
                return masks

            seg_st = {}

            def emit_seg_copy(r):
                # psum -> SBUF bf16 on ACT; emitted one rank late so the
                # copies never head-of-line block mask production
                st = segp.tile([P, 2 * P], BF16, tag=f"segT{r}", name=f"st{r}")
                nc.scalar.copy(st[:, 0:P], seg_ps[(r, 0)])
                nc.vector.tensor_copy(st[:, P : 2 * P], seg_ps[(r, 1)])
                seg_t.append(st)

            for r in range(R):
                masks = emit_masks(r)
                last_rank = r == R - 1
                if last_rank:
                    st = segp.tile([P, 2 * P], BF16, tag=f"segT{r}", name=f"st{r}")
                for gh in range(2):
                    for t in range(NT):
                        nc.tensor.matmul(
                            seg_ps[(r, gh)],
                            masks[t][:, ts(gh, P)],
                            f_t[:, ts(t, P)],
                            start=(t == 0),
                            stop=(t == NT - 1),
                        )
                    if last_rank:
                        # copy each half as soon as its group closes, so the
                        # first output chunk is unblocked sooner
                        if gh == 0:
                            nc.scalar.copy(st[:, 0:P], seg_ps[(r, 0)])
                        else:
                            nc.vector.tensor_copy(st[:, P : 2 * P], seg_ps[(r, 1)])
                if r >= 1:
                    emit_seg_copy(r - 1)
                # stage-2 strip 0 builds at the tail of stage 1, ordered after
                # the LAST rank's stage-1 masks so it cannot steal DVE slots
                # from (and thereby stall) stage-1 mask production
                if r == R - 1:
                    emit_strip_ops(
                        0, range(R), order_dep=last_dve_mask[R - 1], lo=0, hi=256
                    )
                    emit_strip_ops(
                        0, range(R), order_dep=last_dve_mask[R - 1], lo=256, hi=STRIP
                    )
            seg_t.append(st)

        ps_warm_ctx.__exit__(None, None, None)

        # ---- stage-2 output chunks ----

        with tc.tile_pool(name="ps_o", bufs=6, space="PSUM") as ps_o:
            j0 = 0
            for c, w in enumerate(WIDTHS):
                s = j0 // STRIP
                soff = j0 - s * STRIP
                strip = strips[s]
                o_sb = outp.tile([P, w, R + 1], F32, tag="osb", name=f"osb{c}")
                first_chunks = c < 4
                dve_ch = 5 if c % 2 == 0 else 6
                for r in range(R):
                    po = ps_o.tile([P, w], F32, tag="po", name=f"po{c}_{r}")
                    for gh in range(2):
                        nc.tensor.matmul(
                            po[:],
                            seg_t[r][:, ts(gh, P)],
                            strip[(r, gh)][:, soff : soff + w],
                            start=(gh == 0),
                            stop=(gh == 1),
                        )
                    if (first_chunks and r >= 3) or (not first_chunks and r == dve_ch):
                        nc.vector.tensor_copy(o_sb[:, :, r], po[:])
                    else:
                        nc.scalar.copy(o_sb[:, :, r], po[:])
                # identity channel via PE: out[b, j] = f_t[j, b].T
                po = ps_o.tile([P, w], F32, tag="po", name=f"po{c}_id")
                off = 0
                while off < w:
                    j = j0 + off
                    t = j // P
                    seg_w = min(w - off, P - j % P)
                    nc.tensor.matmul(
                        po[:, off : off + seg_w],
                        f_t[:, ts(t, P)],
                        ident_sb[:, j % P : j % P + seg_w],
                        start=True,
                        stop=True,
                    )
                    off += seg_w
                if not first_chunks and dve_ch == 6:
                    nc.vector.tensor_copy(o_sb[:, :, R], po[:])
                else:
                    nc.scalar.copy(o_sb[:, :, R], po[:])
                nc.sync.dma_start(out[:, j0 : j0 + w, :], o_sb[:])
                j0 += w
                # build upcoming strips at fixed points: late enough not to
                # wedge into the stream ramp, early enough that the tail
                # chunks never wait
                for s_ in STRIP_AFTER_CHUNK.get(c, ()):
                    emit_strip_ops(s_, range(R), order_dep=last_strip_op[s_ - 1])

    _legalize_waits(nc)
    return nc


def _host_tables():
    if "tabs_bf" not in _cache:
        iota_g = np.tile(np.arange(G, dtype=np.float64), (P, 1))
        ident = np.eye(P, dtype=np.float64)
        _cache["tabs_bf"] = np.ascontiguousarray(
            np.concatenate([iota_g, ident], axis=1).astype(ml_dtypes.bfloat16)
        )
    return _cache["tabs_bf"]


def kernel(F_genus: np.ndarray, labels: np.ndarray) -> np.ndarray:
    F_genus = np.ascontiguousarray(F_genus, dtype=np.float32)
    labels = np.ascontiguousarray(labels, dtype=np.int32)
    assert F_genus.shape == (B, N) and labels.shape == (R, N)

    tabs_bf = _host_tables()
    # labT[p, r*NT + t] = labels[r, t*128 + p]
    labT = np.transpose(labels.reshape(R, NT, P), (2, 0, 1)).reshape(P, R * NT)
    iota_p = np.arange(P, dtype=np.float64)[:, None] + 128.0 * np.arange(2)[None, :]
    tabs_f32 = np.ascontiguousarray(
        np.concatenate([iota_p, labT], axis=1).astype(np.float32)
    )
    lab_bf = np.ascontiguousarray(labels.astype(np.uint8))

    in_maps = []
    for c in range(NCORES):
        Fc = F_genus[c * BL : (c + 1) * BL]  # [BL, N]
        # f_t[p, t*128 + b] = Fc[b, t*128 + p]
        f_t = np.ascontiguousarray(
            Fc.reshape(BL, NT, P).transpose(2, 1, 0).reshape(P, N)
        ).astype(ml_dtypes.bfloat16)
        in_maps.append(
            {
                "f_t_in": f_t,
                "tabs_bf": tabs_bf,
                "tabs_f32": tabs_f32,
                "lab_bf": lab_bf,
            }
        )

    # The first execution of a freshly compiled NEFF occasionally hits a
    # transient NRT_EXEC_UNIT_UNRECOVERABLE; a rebuild + retry recovers.
    last_err = None
    for attempt in range(3):
        try:
            if "nc" not in _cache:
                _cache["nc"] = _build_nc()
            res = run_bass_kernel_spmd(
                _cache["nc"], in_maps, core_ids=list(range(NCORES))
            )
            return np.concatenate([r["out"] for r in res.results], axis=0)
        except Exception as e:  # noqa: BLE001
            last_err = e
            _cache.pop("nc", None)
            import time as _time

            _time.sleep(3.0)
    raise last_err



# revision 2
# speedup vs baseline: 1.3206x; 1.3206x over previous
"""Trainium2 Bass kernel for nn_Encoder_23124103922122 (segment_reduce), v5.

Math (per rank r of 6, labels lab_r[0..4095] in [0,256)):
    seg_r[b, g]  = sum_{i: lab_r[i]==g} F[b, i]          (segment sum)
    out[b, j, r] = seg_r[b, lab_r[j]]                     (gather back)
    out[b, j, 6] = F[b, j]                                (identity channel)

v5 design — channel-major device output, rank-pipelined:
  - The device computes only the 6 rank channels, laid out channel-major:
    out2[b, r, j] (bf16). The host transposes to [b, j, r] and fills the
    identity channel directly from the f32 input (exact). This removes the
    all-ranks interleave barrier of v2: rank r's channel streams to HBM as
    soon as rank r's stage-2 finishes, and the device output shrinks from
    14.7MB to 6.3MB per core.
  - Per rank: stage-1 runs mask-as-STATIONARY (psum_segT[gh, b] +=
    m1[i, gh].T @ f_t[i, b]), 64 matmuls; stage-2 gathers back via one-hot
    m2[g, j] matmuls, 16 matmuls of 512 moving columns. PE is the wall:
    ~41us of matmul across 6 ranks, software-pipelined one rank deep so
    stage-2 dependency hiccups never stall PE (a spare stage-1 is always
    queued).
  - m1 masks (iota_g vs label scalar, bf16, DVE 4x mode) ~26/rank on DVE +
    6/rank on Pool. m2 masks are two 4096-wide DVE ops per rank comparing a
    u16 label broadcast against the partition iota (4x mode, ~1.1us each).
  - psum->SBUF copies (seg bf16, and the 8 per-rank po->channel copies) run
    on ACT. Channel halves DMA out as their copies land.

Sharding: data-parallel over batch B=1024 -> 8 cores x 128 rows. Labels
replicated. No cross-device communication.

Note: walrus in this container accepts at most ONE sync-wait per instruction
(two on EventSemaphore); _legalize_waits() post-processes the Tile-scheduled
program to satisfy that.
"""

import sys

if "/opt/trn_rl_repo" not in sys.path:
    sys.path.insert(0, "/opt/trn_rl_repo")

from contextlib import ExitStack

import ml_dtypes
import numpy as np

import concourse.bass as bass
import concourse.mybir as mybir
import concourse.tile as tile
from concourse.bass import ts
from concourse.bass_utils import run_bass_kernel_spmd

B, N, R, G = 1024, 4096, 6, 256
NCORES = 8
BL = B // NCORES  # 128 batch rows per core
P = 128
NT = N // P  # 32 genus tiles
F32 = mybir.dt.float32
BF16 = mybir.dt.bfloat16
U16 = mybir.dt.uint16

STRIP = 512  # stage-2 moving width (PE max moving free dim)
NS = N // STRIP  # 8 strips per rank channel

# m1 mask tiles built on Pool (the rest go to DVE)
POOL_T = {5, 11, 17, 23, 29}

_cache: dict = {}

# Engine -> prefix of the semaphore names its compute instructions increment.
# Pool (GPSIMD) is excluded: its 8 DSP cores do not guarantee in-order
# completion, so Pool-on-Pool waits cannot be dropped as redundant.
_ENGINE_SEM_PREFIX = {
    mybir.EngineType.PE: "PE",
    mybir.EngineType.DVE: "DVE",
    mybir.EngineType.Activation: "Activation",
    mybir.EngineType.SP: "SP",
}


def _legalize_waits(nc):
    """Walrus only accepts 1 sync-wait per instruction (2 on EventSemaphore),
    but the Tile scheduler can emit more. Post-pass:
      1. drop waits on the instruction's own engine semaphore that are already
         satisfied by same-engine program order (compute completion is in-order
         and sem targets are absolute), and
      2. hoist remaining excess waits onto EventSemaphore carrier instructions
         inserted just before the instruction on the same engine.
    """
    ev_id = 0
    for f in nc.m.functions:
        for blk in f.blocks:
            insts = blk.instructions
            sem_incs: dict = {}  # (engine, sem_name) -> cumulative inc in stream
            new_insts = []
            for inst in insts:
                si = inst.sync_info
                if si is not None and si.on_wait:
                    cap = 2 if isinstance(inst, mybir.InstEventSemaphore) else 1
                    eng = inst.engine
                    pfx = _ENGINE_SEM_PREFIX.get(eng)
                    kept = []
                    for w in si.on_wait:
                        sem_eng = w.ant_name.rsplit("_", 1)[0]
                        if (
                            pfx is not None
                            and sem_eng == pfx
                            and w.wait_mode == "sem-ge-imm"
                            and sem_incs.get((eng, w.ant_name), 0) >= w.wait_value
                        ):
                            continue  # satisfied by same-engine execution order
                        kept.append(w)
                    while len(kept) > cap:
                        ncarry = min(2, len(kept) - cap + 1)
                        carry, kept = kept[:ncarry], kept[ncarry:]
                        ev = mybir.InstEventSemaphore(
                            name=f"EVW-{ev_id}", ins=[], outs=[]
                        )
                        ev_id += 1
                        ev.engine = eng
                        ev.sync_info = mybir.SyncInfo(on_wait=carry, on_update=[])
                        new_insts.append(ev)
                    inst.sync_info = mybir.SyncInfo(
                        on_wait=kept, on_update=si.on_update
                    )
                si = inst.sync_info
                if si is not None:
                    for u in si.on_update:
                        if u.update_mode == "sem-inc":
                            key = (inst.engine, u.ant_name)
                            sem_incs[key] = sem_incs.get(key, 0) + u.update_value
                new_insts.append(inst)
            if len(new_insts) != len(insts):
                insts[:] = new_insts


def _build_nc():
    nc = bass.Bass("TRN2", debug=False, num_devices=NCORES)

    # f_t[p, t*128 + b] = F[b, t*128 + p]  (bf16 transposed F tiles)
    f_t_in = nc.dram_tensor("f_t_in", [P, N], BF16, kind="ExternalInput").ap()
    # tabs_bf[p, 0:G] = iota_g (bf16)
    tabs_bf = nc.dram_tensor("tabs_bf", [P, G], BF16, kind="ExternalInput").ap()
    # tabs_f32[p, 0:2] = (p, p+128); tabs_f32[p, 2 + r*NT + t] = labels[r, t*128+p]
    tabs_f32 = nc.dram_tensor(
        "tabs_f32", [P, 2 + R * NT], F32, kind="ExternalInput"
    ).ap()
    # lab16[r, j] = labels[r, j] (u16, partition-broadcast source for m2)
    lab16 = nc.dram_tensor("lab16", [R, N], U16, kind="ExternalInput").ap()
    # channel-major output: out2[b, r, j] = seg_r[b, lab_r[j]]
    out2 = nc.dram_tensor("out2", [BL, R, N], BF16, kind="ExternalOutput").ap()

    with ExitStack() as ctx:
        tc = ctx.enter_context(tile.TileContext(nc))

        const = ctx.enter_context(tc.tile_pool(name="const", bufs=1))
        m1p = ctx.enter_context(tc.tile_pool(name="m1p", bufs=66))
        m1pp = ctx.enter_context(tc.tile_pool(name="m1pp", bufs=15))
        m2p = ctx.enter_context(tc.tile_pool(name="m2p", bufs=2))
        segp = ctx.enter_context(tc.tile_pool(name="segp", bufs=3))
        chp = ctx.enter_context(tc.tile_pool(name="chp", bufs=2))
        ps_seg = ctx.enter_context(tc.tile_pool(name="ps_seg", bufs=3, space="PSUM"))
        ps_o = ctx.enter_context(tc.tile_pool(name="ps_o", bufs=4, space="PSUM"))

        # ---- input DMAs. sync (SP) queue: tables, then f_t (compute-
        # critical, in quarters so stage-1 starts early), then the 6 per-rank
        # u16 label broadcasts (rank r's is needed only by its m2 build). ----
        tf32_sb = const.tile([P, 2 + R * NT], F32)
        nc.sync.dma_start(tf32_sb[:], tabs_f32)
        tbf_sb = const.tile([P, G], BF16)
        nc.scalar.dma_start(tbf_sb[:], tabs_bf)
        f_t = const.tile([P, N], BF16)
        NPC = 4
        for q in range(NPC):
            nc.sync.dma_start(
                f_t[:, q * (N // NPC) : (q + 1) * (N // NPC)],
                f_t_in[:, q * (N // NPC) : (q + 1) * (N // NPC)],
            )
        lab_bc = const.tile([P, R, N], U16)
        for r in range(R):
            nc.sync.dma_start(
                lab_bc[:, r : r + 1, :], lab16[r : r + 1, :].partition_broadcast(P)
            )

        iota_p_sb = tf32_sb[:, 0:2]
        labT_sb = tf32_sb[:, 2:]
        iota_g_sb = tbf_sb[:, 0:G]

        # ---- prewarm: absorb const-DMA semaphores into the DVE/Pool clocks
        # (hot-loop ops may carry at most one sync wait), and keep PE busy on
        # junk matmuls until f_t arrives so its p-state ramps. ----
        warm = const.tile([P, 4], BF16)
        nc.vector.tensor_copy(warm[:, 0:1], tf32_sb[:, 0:1])
        nc.vector.tensor_copy(warm[:, 1:2], tbf_sb[:, 0:1])
        nc.gpsimd.tensor_copy(warm[:, 2:3], tf32_sb[:, 1:2])
        nc.gpsimd.tensor_copy(warm[:, 3:4], tbf_sb[:, 1:2])
        with tc.tile_pool(name="ps_warm", bufs=1, space="PSUM") as ps_warm:
            wps = ps_warm.tile([P, P], F32)
            wjunk = const.tile([P, P], BF16)
            nc.vector.memset(wjunk[:], 0.0)
            # p-state warm-up on an UNINITIALIZED tile: no input dependency,
            # so PE is busy from ~0.4us (values are zero and never read)
            for _ in range(18):
                nc.tensor.matmul(wps[:], wjunk[:], wjunk[:], start=True, stop=True)

        # ---- per-rank emitters ----
        seg_ps = {}
        seg_t = {}
        m2 = {}

        def emit_m1(r):
            masks = []
            for t in range(NT):
                col = r * NT + t
                if t in POOL_T:
                    mt = m1pp.tile([P, G], BF16, tag="m1p", name=f"m1p{r}_{t}")
                    eng = nc.gpsimd
                else:
                    mt = m1p.tile([P, G], BF16, tag="m1", name=f"m1_{r}_{t}")
                    eng = nc.vector
                eng.tensor_scalar(
                    mt[:],
                    iota_g_sb[:],
                    labT_sb[:, col : col + 1],
                    None,
                    op0=mybir.AluOpType.is_equal,
                )
                masks.append(mt)
            return masks

        def emit_m2(r):
            # m2[gh*128+g, j] = (lab_r[j] == gh*128+g); two 4096-wide DVE ops
            # (u16 vs f32 scalar, bf16 out -> 4x mode)
            t_ = m2p.tile([P, 2, N], BF16, tag="m2", name=f"m2_{r}")
            for gh in range(2):
                nc.vector.tensor_scalar(
                    t_[:, gh, :],
                    lab_bc[:, r, :],
                    iota_p_sb[:, gh : gh + 1],
                    None,
                    op0=mybir.AluOpType.is_equal,
                )
            m2[r] = t_

        def emit_s1(r, masks):
            t_ = ps_seg.tile([P, 2, P], F32, tag="segps", name=f"segps{r}")
            seg_ps[r] = t_
            for gh in range(2):
                for t in range(NT):
                    nc.tensor.matmul(
                        t_[:, gh, :],
                        masks[t][:, ts(gh, P)],
                        f_t[:, ts(t, P)],
                        start=(t == 0),
                        stop=(t == NT - 1),
                    )

        def emit_seg_copy(r):
            st = segp.tile([P, 2, P], BF16, tag="segT", name=f"st{r}")
            nc.scalar.copy(st[:, 0, :], seg_ps[r][:, 0, :])
            nc.scalar.copy(st[:, 1, :], seg_ps[r][:, 1, :])
            seg_t[r] = st

        def emit_s2(r):
            # stage-2 + po->channel copies + channel DMA (in halves)
            ch = chp.tile([P, N], BF16, tag="ch", name=f"ch{r}")
            for s in range(NS):
                po = ps_o.tile([P, STRIP], F32, tag="po", name=f"po{r}_{s}")
                for gh in range(2):
                    nc.tensor.matmul(
                        po[:],
                        seg_t[r][:, gh, :],
                        m2[r][:, gh, ts(s, STRIP)],
                        start=(gh == 0),
                        stop=(gh == 1),
                    )
                nc.scalar.copy(ch[:, ts(s, STRIP)], po[:])
                if r == R - 1:
                    # fine-grained tail: stream the last channel per strip
                    nc.sync.dma_start(out2[:, r, ts(s, STRIP)], ch[:, ts(s, STRIP)])
            if r < R - 1:
                for h in range(2):
                    nc.sync.dma_start(
                        out2[:, r, ts(h, N // 2)], ch[:, ts(h, N // 2)]
                    )

        # ---- software-pipelined emission: PE order is
        # s1(0) s1(1) s2(0) s1(2) s2(1) ... s1(5) s2(4) s2(5), so PE always
        # has a stage-1 queued while stage-2 deps (m2, seg copies) settle. ----
        masks0 = emit_m1(0)
        emit_m2(0)
        emit_s1(0, masks0)
        emit_seg_copy(0)
        masks1 = emit_m1(1)
        emit_m2(1)
        emit_s1(1, masks1)
        emit_seg_copy(1)
        for r in range(R):
            emit_s2(r)
            if r + 2 < R:
                masks = emit_m1(r + 2)
                emit_m2(r + 2)
                emit_s1(r + 2, masks)
                emit_seg_copy(r + 2)

    _legalize_waits(nc)
    return nc


def _host_tables():
    if "tabs_bf" not in _cache:
        _cache["tabs_bf"] = np.ascontiguousarray(
            np.tile(np.arange(G, dtype=np.float64), (P, 1)).astype(ml_dtypes.bfloat16)
        )
    return _cache["tabs_bf"]


def kernel(F_genus: np.ndarray, labels: np.ndarray) -> np.ndarray:
    F_genus = np.ascontiguousarray(F_genus, dtype=np.float32)
    labels = np.ascontiguousarray(labels, dtype=np.int32)
    assert F_genus.shape == (B, N) and labels.shape == (R, N)

    tabs_bf = _host_tables()
    # labT[p, r*NT + t] = labels[r, t*128 + p]
    labT = np.transpose(labels.reshape(R, NT, P), (2, 0, 1)).reshape(P, R * NT)
    iota_p = np.arange(P, dtype=np.float64)[:, None] + 128.0 * np.arange(2)[None, :]
    tabs_f32 = np.ascontiguousarray(
        np.concatenate([iota_p, labT], axis=1).astype(np.float32)
    )
    lab16 = np.ascontiguousarray(labels.astype(np.uint16))

    in_maps = []
    for c in range(NCORES):
        Fc = F_genus[c * BL : (c + 1) * BL]  # [BL, N]
        # f_t[p, t*128 + b] = Fc[b, t*128 + p]
        f_t = np.ascontiguousarray(
            Fc.reshape(BL, NT, P).transpose(2, 1, 0).reshape(P, N)
        ).astype(ml_dtypes.bfloat16)
        in_maps.append(
            {
                "f_t_in": f_t,
                "tabs_bf": tabs_bf,
                "tabs_f32": tabs_f32,
                "lab16": lab16,
            }
        )

    # The first execution of a freshly compiled NEFF occasionally hits a
    # transient NRT_EXEC_UNIT_UNRECOVERABLE; a rebuild + retry recovers.
    last_err = None
    for attempt in range(3):
        try:
            if "nc" not in _cache:
                _cache["nc"] = _build_nc()
            res = run_bass_kernel_spmd(
                _cache["nc"], in_maps, core_ids=list(range(NCORES))
            )
            out = np.empty((B, N, R + 1), dtype=np.float32)
            for c in range(NCORES):
                # out2 is [BL, R, N] bf16, channel-major -> transpose
                out[c * BL : (c + 1) * BL, :, :R] = (
                    res.results[c]["out2"].astype(np.float32).transpose(0, 2, 1)
                )
            out[:, :, R] = F_genus  # identity channel, exact
            return out
        except Exception as e:  # noqa: BLE001
            last_err = e
            _cache.pop("nc", None)
            import time as _time

            _time.sleep(3.0)
    raise last_err


# revision 27
# speedup vs baseline: 1.5695x; 1.1885x over previous
"""Trainium2 Bass kernel for nn_Encoder_23124103922122 (segment_reduce), v5.

Math (per rank r of 6, labels lab_r[0..4095] in [0,256)):
    seg_r[b, g]  = sum_{i: lab_r[i]==g} F[b, i]          (segment sum)
    out[b, j, r] = seg_r[b, lab_r[j]]                     (gather back)
    out[b, j, 6] = F[b, j]                                (identity channel)

v5 design — channel-major device output, rank-pipelined:
  - The device computes only the 6 rank channels, laid out channel-major:
    out2[b, r, j] (bf16). The host transposes to [b, j, r] and fills the
    identity channel directly from the f32 input (exact). This removes the
    all-ranks interleave barrier of v2: rank r's channel streams to HBM as
    soon as rank r's stage-2 finishes, and the device output shrinks from
    14.7MB to 6.3MB per core.
  - Per rank: stage-1 runs mask-as-STATIONARY (psum_segT[gh, b] +=
    m1[i, gh].T @ f_t[i, b]), 64 matmuls; stage-2 gathers back via one-hot
    m2[g, j] matmuls, 16 matmuls of 512 moving columns. PE is the wall:
    ~41us of matmul across 6 ranks, software-pipelined one rank deep so
    stage-2 dependency hiccups never stall PE (a spare stage-1 is always
    queued).
  - m1 masks (iota_g vs label scalar, bf16, DVE 4x mode) ~26/rank on DVE +
    6/rank on Pool. m2 masks are two 4096-wide DVE ops per rank comparing a
    u16 label broadcast against the partition iota (4x mode, ~1.1us each).
  - psum->SBUF copies (seg bf16, and the 8 per-rank po->channel copies) run
    on ACT. Channel halves DMA out as their copies land.

Sharding: data-parallel over batch B=1024 -> 8 cores x 128 rows. Labels
replicated. No cross-device communication.

Note: walrus in this container accepts at most ONE sync-wait per instruction
(two on EventSemaphore); _legalize_waits() post-processes the Tile-scheduled
program to satisfy that.
"""

import sys

if "/opt/trn_rl_repo" not in sys.path:
    sys.path.insert(0, "/opt/trn_rl_repo")

from contextlib import ExitStack

import ml_dtypes
import numpy as np

import concourse.bass as bass
import concourse.mybir as mybir
import concourse.tile as tile
from concourse.bass import ts
from concourse.bass_utils import run_bass_kernel_spmd

B, N, R, G = 1024, 4096, 6, 256
NCORES = 8
BL = B // NCORES  # 128 batch rows per core
P = 128
NT = N // P  # 32 genus tiles
F32 = mybir.dt.float32
BF16 = mybir.dt.bfloat16
U16 = mybir.dt.uint16
FP8 = mybir.dt.float8e4

STRIP = 512  # stage-2 moving width (PE max moving free dim)
NS = N // STRIP  # 8 strips per rank channel

# m1 mask tiles built on Pool (the rest go to DVE). Pool is ~4.2x slower
# per tile, so its tiles sit late in the rank (PE reaches them last) and the
# ramp ranks (whose windows are half-length) get fewer of them.
POOL_T_RAMP = {13, 19, 25, 31}
POOL_T = {9, 12, 14, 16, 18, 20, 22, 24, 26, 29, 31}
# stage-2 strips whose po->channel copy runs on DVE (rest on ACT)
DVE_S = {2, 5}

_cache: dict = {}

# Engine -> prefix of the semaphore names its compute instructions increment.
# Pool (GPSIMD) is excluded: its 8 DSP cores do not guarantee in-order
# completion, so Pool-on-Pool waits cannot be dropped as redundant.
_ENGINE_SEM_PREFIX = {
    mybir.EngineType.PE: "PE",
    mybir.EngineType.DVE: "DVE",
    mybir.EngineType.Activation: "Activation",
    mybir.EngineType.SP: "SP",
}


def _legalize_waits(nc):
    """Walrus only accepts 1 sync-wait per instruction (2 on EventSemaphore),
    but the Tile scheduler can emit more. Post-pass:
      1. drop waits on the instruction's own engine semaphore that are already
         satisfied by same-engine program order (compute completion is in-order
         and sem targets are absolute), and
      2. hoist remaining excess waits onto EventSemaphore carrier instructions
         inserted just before the instruction on the same engine.
    """
    ev_id = 0
    for f in nc.m.functions:
        for blk in f.blocks:
            insts = blk.instructions
            sem_incs: dict = {}  # (engine, sem_name) -> cumulative inc in stream
            new_insts = []
            for inst in insts:
                si = inst.sync_info
                if si is not None and si.on_wait:
                    cap = 2 if isinstance(inst, mybir.InstEventSemaphore) else 1
                    eng = inst.engine
                    pfx = _ENGINE_SEM_PREFIX.get(eng)
                    kept = []
                    for w in si.on_wait:
                        sem_eng = w.ant_name.rsplit("_", 1)[0]
                        if (
                            pfx is not None
                            and sem_eng == pfx
                            and w.wait_mode == "sem-ge-imm"
                            and sem_incs.get((eng, w.ant_name), 0) >= w.wait_value
                        ):
                            continue  # satisfied by same-engine execution order
                        kept.append(w)
                    while len(kept) > cap:
                        ncarry = min(2, len(kept) - cap + 1)
                        carry, kept = kept[:ncarry], kept[ncarry:]
                        ev = mybir.InstEventSemaphore(
                            name=f"EVW-{ev_id}", ins=[], outs=[]
                        )
                        ev_id += 1
                        ev.engine = eng
                        ev.sync_info = mybir.SyncInfo(on_wait=carry, on_update=[])
                        new_insts.append(ev)
                    inst.sync_info = mybir.SyncInfo(
                        on_wait=kept, on_update=si.on_update
                    )
                si = inst.sync_info
                if si is not None:
                    for u in si.on_update:
                        if u.update_mode == "sem-inc":
                            key = (inst.engine, u.ant_name)
                            sem_incs[key] = sem_incs.get(key, 0) + u.update_value
                new_insts.append(inst)
            if len(new_insts) != len(insts):
                insts[:] = new_insts


def _build_nc():
    nc = bass.Bass("TRN2", debug=False, num_devices=NCORES)

    # f_t[p, t*128 + b] = F[b, t*128 + p]  (bf16 transposed F tiles)
    f_t_in = nc.dram_tensor("f_t_in", [P, N], BF16, kind="ExternalInput").ap()
    # tabs16[p, 0:G] = iota_g; [G:G+2] = (p, p+128); [G+2+r*NT+t] =
    # labels[r, t*128+p] -- one u16 table tensor, one DMA
    tabs16 = nc.dram_tensor(
        "tabs16", [P, G + 2 + R * NT], U16, kind="ExternalInput"
    ).ap()
    # m28[r, g, h, j] = (labels[r, j] == h*128 + g), fp8e4 (host-built
    # one-hot gather masks, DoubleRow-packed: h is the k-tile axis)
    m28_in = nc.dram_tensor("m28", [R, P, 2, N], FP8, kind="ExternalInput").ap()
    # channel-major output: out2[b, r, j] = seg_r[b, lab_r[j]]
    out2 = nc.dram_tensor("out2", [BL, R, N], BF16, kind="ExternalOutput").ap()

    with ExitStack() as ctx:
        tc = ctx.enter_context(tile.TileContext(nc))

        const = ctx.enter_context(tc.tile_pool(name="const", bufs=1))
        m1p = ctx.enter_context(tc.tile_pool(name="m1p", bufs=52))
        m1pp = ctx.enter_context(tc.tile_pool(name="m1pp", bufs=22))
        segp = ctx.enter_context(tc.tile_pool(name="segp", bufs=3))
        chp = ctx.enter_context(tc.tile_pool(name="chp", bufs=3))
        ps_seg = ctx.enter_context(tc.tile_pool(name="ps_seg", bufs=2, space="PSUM"))
        ps_o = ctx.enter_context(tc.tile_pool(name="ps_o", bufs=5, space="PSUM"))

        # ---- input DMAs. sync (SP) queue: tables, then f_t (compute-
        # critical, in quarters so stage-1 starts early), then the 6 per-rank
        # u16 label broadcasts (rank r's is needed only by its m2 build). ----
        t16_sb = const.tile([P, G + 2 + R * NT], U16)
        nc.sync.dma_start(t16_sb[:], tabs16)
        f_t = const.tile([P, N], BF16)
        j0 = 0
        for cw in (256, 384, 512, 640, 768, 896, 640):
            nc.sync.dma_start(f_t[:, j0 : j0 + cw], f_t_in[:, j0 : j0 + cw])
            j0 += cw
        m2_sb = const.tile([P, R, 2, N], FP8)
        for r in range(R):
            nc.sync.dma_start(m2_sb[:, r], m28_in[r])

        iota_g_sb = t16_sb[:, 0:G]
        # is_equal scalars must be f32: convert the iota_p/labT columns once
        scal_sb = const.tile([P, 2 + R * NT], F32)
        nc.scalar.copy(scal_sb[:], t16_sb[:, G:])
        iota_p_sb = scal_sb[:, 0:2]
        labT_sb = scal_sb[:, 2:]

        # ---- prewarm: absorb const-DMA semaphores into the DVE/Pool clocks
        # (hot-loop ops may carry at most one sync wait), and keep PE busy on
        # junk matmuls until f_t arrives so its p-state ramps. ----
        warm = const.tile([P, 4], BF16)
        wjunk = const.tile([P, P], BF16)
        nc.vector.memset(wjunk[:], 0.0)
        nc.vector.tensor_copy(warm[:, 0:1], t16_sb[:, 0:1])
        nc.gpsimd.tensor_copy(warm[:, 2:3], t16_sb[:, 1:2])
        with tc.tile_pool(name="ps_warm", bufs=1, space="PSUM") as ps_warm:
            wps = ps_warm.tile([P, P], F32)
            # p-state warm-up on UNINITIALIZED tiles: no input dependency, so
            # PE is busy from ~0.6us (the product is never read)
            for _ in range(28):
                nc.tensor.matmul(wps[:], wjunk[:], wjunk[:], start=True, stop=True)

        # ---- per-rank emitters ----
        seg_ps = {}
        seg_t = {}

        def emit_m1(r):
            masks = []
            pool_t = POOL_T_RAMP if r < 2 else POOL_T
            for t in range(NT):
                col = r * NT + t
                if t in pool_t:
                    mt = m1pp.tile([P, G], BF16, tag="m1p", name=f"m1p{r}_{t}")
                    eng = nc.gpsimd
                else:
                    mt = m1p.tile([P, G], BF16, tag="m1", name=f"m1_{r}_{t}")
                    eng = nc.vector
                eng.tensor_scalar(
                    mt[:],
                    iota_g_sb[:],
                    labT_sb[:, col : col + 1],
                    None,
                    op0=mybir.AluOpType.is_equal,
                )
                masks.append(mt)
            return masks

        def emit_s1(r, masks):
            # gh outer / t inner: interleaved accumulation groups miscompute
            # on HW (probe-verified), so groups must stay contiguous
            t_ = ps_seg.tile([P, 2, P], F32, tag="segps", name=f"segps{r}")
            seg_ps[r] = t_
            for gh in range(2):
                for t in range(NT):
                    nc.tensor.matmul(
                        t_[:, gh, :],
                        masks[t][:, ts(gh, P)],
                        f_t[:, ts(t, P)],
                        start=(t == 0),
                        stop=(t == NT - 1),
                    )

        def emit_seg_copy(r):
            # seg -> fp8 hi (seg8) + fp8 residual (res8); seg8 + res8
            # reconstructs seg to ~2^-8 relative. All ops same-dtype pairs.
            sbf = segp.tile([P, 2, P], BF16, tag="sbf", name=f"sbf{r}")
            nc.vector.tensor_copy(sbf[:], seg_ps[r][:])
            s8 = segp.tile([P, 2, P], FP8, tag="s8", name=f"s8_{r}")
            nc.scalar.copy(s8[:], seg_ps[r][:])
            s8b = segp.tile([P, 2, P], BF16, tag="s8b", name=f"s8b{r}")
            nc.scalar.copy(s8b[:], s8[:])
            rbf = segp.tile([P, 2, P], BF16, tag="rbf", name=f"rbf{r}")
            nc.vector.tensor_tensor(
                rbf[:], sbf[:], s8b[:], op=mybir.AluOpType.subtract
            )
            r8 = segp.tile([P, 2, P], FP8, tag="r8", name=f"r8_{r}")
            nc.scalar.copy(r8[:], rbf[:])
            seg_t[r] = (s8, r8)

        ch_of = {}
        W2 = 256  # DoubleRow strip width (rhs free = 2*W2 = PE moving max)

        def emit_s2(r, p_lo=0, p_hi=8):
            # stage-2: per 256-strip, two fp8 DoubleRow matmuls (seg8 then
            # res8) contract all 256 groups at 0.5 cyc/row. Strips pair up in
            # one psum tile so each po->channel copy moves 512 columns.
            if r in ch_of:
                ch = ch_of[r]
            else:
                ch = chp.tile([P, N], BF16, tag="ch", name=f"ch{r}")
                ch_of[r] = ch
            s8, r8 = seg_t[r]
            for p_ in range(p_lo, p_hi):
                po = ps_o.tile([P, 2, W2], F32, tag="po", name=f"po{r}_{p_}")
                for half in range(2):
                    s = 2 * p_ + half
                    for op8 in (s8, r8):
                        nc.tensor.matmul(
                            po[:, half, :],
                            op8[:],
                            m2_sb[:, r, :, ts(s, W2)],
                            start=(op8 is s8),
                            stop=(op8 is r8),
                            perf_mode=mybir.MatmulPerfMode.DoubleRow,
                        )
                dve = p_ % 2 == 1 if r >= R - 2 else p_ % 4 == 2
                if dve:
                    nc.vector.tensor_copy(ch[:, ts(p_, 2 * W2)], po[:])
                else:
                    nc.scalar.copy(ch[:, ts(p_, 2 * W2)], po[:])
                # stream in quarters (2 pairs each): short final drain,
                # cheap per-DMA HWDGE hold (625ns, exclusive)
                if p_ % 2 == 1:
                    q = p_ // 2
                    nc.sync.dma_start(
                        out2[:, r, ts(q, N // 4)], ch[:, ts(q, N // 4)]
                    )

        # ---- software-pipelined emission: PE order is
        # s1(0) s1(1) s2(0) s1(2) s2(1) ... s1(5) s2(4) s2(5), so PE always
        # has a stage-1 queued while stage-2 deps (m2, seg copies) settle.
        # m2(r+1) is emitted during s2(r)'s window so DVE never idles on a
        # label DMA and each m2 is ready one full window early. ----
        masks0 = emit_m1(0)
        emit_s1(0, masks0)
        emit_seg_copy(0)
        masks1 = emit_m1(1)
        emit_s1(1, masks1)
        emit_seg_copy(1)
        for r in range(4):
            emit_s2(r)
            masks = emit_m1(r + 2)
            emit_s1(r + 2, masks)
            emit_seg_copy(r + 2)
        # tail: interleave ranks 4/5 stage-2 halves so rank 5's seg prep
        # settles during rank 4's strips and the final drain is ~0.5MB
        emit_s2(4, 0, 6)
        emit_s2(5, 0, 6)
        emit_s2(4, 6, 8)
        emit_s2(5, 6, 8)

    _legalize_waits(nc)
    return nc


def _host_tables(labels):
    # tabs16 = [iota_g | iota_p, iota_p+128 | labT] as u16
    iota_g = np.tile(np.arange(G, dtype=np.uint16), (P, 1))
    iota_p = (
        np.arange(P, dtype=np.uint16)[:, None]
        + np.uint16(128) * np.arange(2, dtype=np.uint16)[None, :]
    )
    labT = (
        np.transpose(labels.reshape(R, NT, P), (2, 0, 1))
        .reshape(P, R * NT)
        .astype(np.uint16)
    )
    return np.ascontiguousarray(
        np.concatenate([iota_g, iota_p, labT], axis=1).astype(np.uint16)
    )


def kernel(F_genus: np.ndarray, labels: np.ndarray) -> np.ndarray:
    F_genus = np.ascontiguousarray(F_genus, dtype=np.float32)
    labels = np.ascontiguousarray(labels, dtype=np.int32)
    assert F_genus.shape == (B, N) and labels.shape == (R, N)

    tabs16 = _host_tables(labels)
    # m28[r, g, h, j] = (labels[r, j] == h*128 + g) in fp8e4 (exact 0/1)
    gvals = (
        np.arange(P, dtype=np.int32)[None, :, None, None]
        + 128 * np.arange(2, dtype=np.int32)[None, None, :, None]
    )
    m28 = np.ascontiguousarray(
        (labels[:, None, None, :] == gvals).astype(ml_dtypes.float8_e4m3fn)
    )

    in_maps = []
    for c in range(NCORES):
        Fc = F_genus[c * BL : (c + 1) * BL]  # [BL, N]
        # f_t[p, t*128 + b] = Fc[b, t*128 + p]
        f_t = np.ascontiguousarray(
            Fc.reshape(BL, NT, P).transpose(2, 1, 0).reshape(P, N)
        ).astype(ml_dtypes.bfloat16)
        in_maps.append(
            {
                "f_t_in": f_t,
                "tabs16": tabs16,
                "m28": m28,
            }
        )

    # The first execution of a freshly compiled NEFF occasionally hits a
    # transient NRT_EXEC_UNIT_UNRECOVERABLE; a rebuild + retry recovers.
    last_err = None
    for attempt in range(3):
        try:
            if "nc" not in _cache:
                _cache["nc"] = _build_nc()
            res = run_bass_kernel_spmd(
                _cache["nc"], in_maps, core_ids=list(range(NCORES))
            )
            out = np.empty((B, N, R + 1), dtype=np.float32)
            for c in range(NCORES):
                # out2 is [BL, R, N] bf16, channel-major -> transpose
                out[c * BL : (c + 1) * BL, :, :R] = (
                    res.results[c]["out2"].astype(np.float32).transpose(0, 2, 1)
                )
            out[:, :, R] = F_genus  # identity channel, exact
            return out
        except Exception as e:  # noqa: BLE001
            last_err = e
            _cache.pop("nc", None)
            import time as _time

            _time.sleep(3.0)
    raise last_err


# revision 30
# speedup vs baseline: 1.5784x; 1.0057x over previous
"""Trainium2 Bass kernel for nn_Encoder_23124103922122 (segment_reduce), v5.

Math (per rank r of 6, labels lab_r[0..4095] in [0,256)):
    seg_r[b, g]  = sum_{i: lab_r[i]==g} F[b, i]          (segment sum)
    out[b, j, r] = seg_r[b, lab_r[j]]                     (gather back)
    out[b, j, 6] = F[b, j]                                (identity channel)

v5 design — channel-major device output, rank-pipelined:
  - The device computes only the 6 rank channels, laid out channel-major:
    out2[b, r, j] (bf16). The host transposes to [b, j, r] and fills the
    identity channel directly from the f32 input (exact). This removes the
    all-ranks interleave barrier of v2: rank r's channel streams to HBM as
    soon as rank r's stage-2 finishes, and the device output shrinks from
    14.7MB to 6.3MB per core.
  - Per rank: stage-1 runs mask-as-STATIONARY (psum_segT[gh, b] +=
    m1[i, gh].T @ f_t[i, b]), 64 matmuls; stage-2 gathers back via one-hot
    m2[g, j] matmuls, 16 matmuls of 512 moving columns. PE is the wall:
    ~41us of matmul across 6 ranks, software-pipelined one rank deep so
    stage-2 dependency hiccups never stall PE (a spare stage-1 is always
    queued).
  - m1 masks (iota_g vs label scalar, bf16, DVE 4x mode) ~26/rank on DVE +
    6/rank on Pool. m2 masks are two 4096-wide DVE ops per rank comparing a
    u16 label broadcast against the partition iota (4x mode, ~1.1us each).
  - psum->SBUF copies (seg bf16, and the 8 per-rank po->channel copies) run
    on ACT. Channel halves DMA out as their copies land.

Sharding: data-parallel over batch B=1024 -> 8 cores x 128 rows. Labels
replicated. No cross-device communication.

Note: walrus in this container accepts at most ONE sync-wait per instruction
(two on EventSemaphore); _legalize_waits() post-processes the Tile-scheduled
program to satisfy that.
"""

import sys

if "/opt/trn_rl_repo" not in sys.path:
    sys.path.insert(0, "/opt/trn_rl_repo")

from contextlib import ExitStack

import ml_dtypes
import numpy as np

import concourse.bass as bass
import concourse.mybir as mybir
import concourse.tile as tile
from concourse.bass import ts
from concourse.bass_utils import run_bass_kernel_spmd

B, N, R, G = 1024, 4096, 6, 256
NCORES = 8
BL = B // NCORES  # 128 batch rows per core
P = 128
NT = N // P  # 32 genus tiles
F32 = mybir.dt.float32
BF16 = mybir.dt.bfloat16
U16 = mybir.dt.uint16
FP8 = mybir.dt.float8e4

STRIP = 512  # stage-2 moving width (PE max moving free dim)
NS = N // STRIP  # 8 strips per rank channel

# m1 mask tiles built on Pool (the rest go to DVE). Pool is ~4.2x slower
# per tile, so its tiles sit late in the rank (PE reaches them last) and the
# ramp ranks (whose windows are half-length) get fewer of them.
POOL_T_RAMP = {13, 19, 25, 31}
POOL_T = {9, 12, 14, 16, 18, 20, 22, 24, 26, 29, 31}
# stage-2 strips whose po->channel copy runs on DVE (rest on ACT)
DVE_S = {2, 5}

_cache: dict = {}

# Engine -> prefix of the semaphore names its compute instructions increment.
# Pool (GPSIMD) is excluded: its 8 DSP cores do not guarantee in-order
# completion, so Pool-on-Pool waits cannot be dropped as redundant.
_ENGINE_SEM_PREFIX = {
    mybir.EngineType.PE: "PE",
    mybir.EngineType.DVE: "DVE",
    mybir.EngineType.Activation: "Activation",
    mybir.EngineType.SP: "SP",
}


def _legalize_waits(nc):
    """Walrus only accepts 1 sync-wait per instruction (2 on EventSemaphore),
    but the Tile scheduler can emit more. Post-pass:
      1. drop waits on the instruction's own engine semaphore that are already
         satisfied by same-engine program order (compute completion is in-order
         and sem targets are absolute), and
      2. hoist remaining excess waits onto EventSemaphore carrier instructions
         inserted just before the instruction on the same engine.
    """
    ev_id = 0
    for f in nc.m.functions:
        for blk in f.blocks:
            insts = blk.instructions
            sem_incs: dict = {}  # (engine, sem_name) -> cumulative inc in stream
            new_insts = []
            for inst in insts:
                si = inst.sync_info
                if si is not None and si.on_wait:
                    cap = 2 if isinstance(inst, mybir.InstEventSemaphore) else 1
                    eng = inst.engine
                    pfx = _ENGINE_SEM_PREFIX.get(eng)
                    kept = []
                    for w in si.on_wait:
                        sem_eng = w.ant_name.rsplit("_", 1)[0]
                        if (
                            pfx is not None
                            and sem_eng == pfx
                            and w.wait_mode == "sem-ge-imm"
                            and sem_incs.get((eng, w.ant_name), 0) >= w.wait_value
                        ):
                            continue  # satisfied by same-engine execution order
                        kept.append(w)
                    while len(kept) > cap:
                        ncarry = min(2, len(kept) - cap + 1)
                        carry, kept = kept[:ncarry], kept[ncarry:]
                        ev = mybir.InstEventSemaphore(
                            name=f"EVW-{ev_id}", ins=[], outs=[]
                        )
                        ev_id += 1
                        ev.engine = eng
                        ev.sync_info = mybir.SyncInfo(on_wait=carry, on_update=[])
                        new_insts.append(ev)
                    inst.sync_info = mybir.SyncInfo(
                        on_wait=kept, on_update=si.on_update
                    )
                si = inst.sync_info
                if si is not None:
                    for u in si.on_update:
                        if u.update_mode == "sem-inc":
                            key = (inst.engine, u.ant_name)
                            sem_incs[key] = sem_incs.get(key, 0) + u.update_value
                new_insts.append(inst)
            if len(new_insts) != len(insts):
                insts[:] = new_insts


def _build_nc():
    nc = bass.Bass("TRN2", debug=False, num_devices=NCORES)

    # f_t[p, t*128 + b] = F[b, t*128 + p]  (bf16 transposed F tiles)
    f_t_in = nc.dram_tensor("f_t_in", [P, N], BF16, kind="ExternalInput").ap()
    # tabs16[p, 0:G] = iota_g; [G:G+2] = (p, p+128); [G+2+r*NT+t] =
    # labels[r, t*128+p] -- one u16 table tensor, one DMA
    tabs16 = nc.dram_tensor(
        "tabs16", [P, G + 2 + R * NT], U16, kind="ExternalInput"
    ).ap()
    # m28[r, g, h, j] = (labels[r, j] == h*128 + g), fp8e4 (host-built
    # one-hot gather masks, DoubleRow-packed: h is the k-tile axis)
    m28_in = nc.dram_tensor("m28", [R, P, 2, N], FP8, kind="ExternalInput").ap()
    # channel-major output: out2[b, r, j] = seg_r[b, lab_r[j]]
    out2 = nc.dram_tensor("out2", [BL, R, N], BF16, kind="ExternalOutput").ap()

    with ExitStack() as ctx:
        tc = ctx.enter_context(tile.TileContext(nc))

        const = ctx.enter_context(tc.tile_pool(name="const", bufs=1))
        m1p = ctx.enter_context(tc.tile_pool(name="m1p", bufs=52))
        m1pp = ctx.enter_context(tc.tile_pool(name="m1pp", bufs=22))
        segp = ctx.enter_context(tc.tile_pool(name="segp", bufs=3))
        chp = ctx.enter_context(tc.tile_pool(name="chp", bufs=3))
        ps_seg = ctx.enter_context(tc.tile_pool(name="ps_seg", bufs=2, space="PSUM"))
        ps_o = ctx.enter_context(tc.tile_pool(name="ps_o", bufs=5, space="PSUM"))

        # ---- input DMAs. sync (SP) queue: tables, then f_t (compute-
        # critical, in quarters so stage-1 starts early), then the 6 per-rank
        # u16 label broadcasts (rank r's is needed only by its m2 build). ----
        t16_sb = const.tile([P, G + 2 + R * NT], U16)
        nc.sync.dma_start(t16_sb[:], tabs16)
        f_t = const.tile([P, N], BF16)
        j0 = 0
        for cw in (256, 384, 512, 640, 768, 896, 640):
            nc.sync.dma_start(f_t[:, j0 : j0 + cw], f_t_in[:, j0 : j0 + cw])
            j0 += cw
        m2_sb = const.tile([P, R, 2, N], FP8)
        for r in range(R):
            nc.sync.dma_start(m2_sb[:, r], m28_in[r])

        iota_g_sb = t16_sb[:, 0:G]
        # is_equal scalars must be f32: convert the iota_p/labT columns once
        scal_sb = const.tile([P, 2 + R * NT], F32)
        nc.scalar.copy(scal_sb[:], t16_sb[:, G:])
        iota_p_sb = scal_sb[:, 0:2]
        labT_sb = scal_sb[:, 2:]

        # ---- prewarm: absorb const-DMA semaphores into the DVE/Pool clocks
        # (hot-loop ops may carry at most one sync wait), and keep PE busy on
        # junk matmuls until f_t arrives so its p-state ramps. ----
        warm = const.tile([P, 4], BF16)
        wjunk = const.tile([P, P], BF16)
        nc.vector.memset(wjunk[:], 0.0)
        nc.vector.tensor_copy(warm[:, 0:1], t16_sb[:, 0:1])
        nc.gpsimd.tensor_copy(warm[:, 2:3], t16_sb[:, 1:2])
        with tc.tile_pool(name="ps_warm", bufs=1, space="PSUM") as ps_warm:
            wps = ps_warm.tile([P, P], F32)
            # p-state warm-up on UNINITIALIZED tiles: no input dependency, so
            # PE is busy from ~0.6us (the product is never read)
            for _ in range(28):
                nc.tensor.matmul(wps[:], wjunk[:], wjunk[:], start=True, stop=True)

        # ---- per-rank emitters ----
        seg_ps = {}
        seg_t = {}

        def emit_m1(r):
            masks = []
            pool_t = POOL_T_RAMP if r < 2 else POOL_T
            for t in range(NT):
                col = r * NT + t
                if t in pool_t:
                    mt = m1pp.tile([P, G], BF16, tag="m1p", name=f"m1p{r}_{t}")
                    eng = nc.gpsimd
                else:
                    mt = m1p.tile([P, G], BF16, tag="m1", name=f"m1_{r}_{t}")
                    eng = nc.vector
                eng.tensor_scalar(
                    mt[:],
                    iota_g_sb[:],
                    labT_sb[:, col : col + 1],
                    None,
                    op0=mybir.AluOpType.is_equal,
                )
                masks.append(mt)
            return masks

        def emit_s1(r, masks):
            # gh outer / t inner: interleaved accumulation groups miscompute
            # on HW (probe-verified), so groups must stay contiguous
            t_ = ps_seg.tile([P, 2, P], F32, tag="segps", name=f"segps{r}")
            seg_ps[r] = t_
            for gh in range(2):
                for t in range(NT):
                    nc.tensor.matmul(
                        t_[:, gh, :],
                        masks[t][:, ts(gh, P)],
                        f_t[:, ts(t, P)],
                        start=(t == 0),
                        stop=(t == NT - 1),
                    )

        def emit_seg_copy(r):
            # seg -> fp8 hi (seg8) + fp8 residual (res8); seg8 + res8
            # reconstructs seg to ~2^-8 relative. All ops same-dtype pairs.
            sbf = segp.tile([P, 2, P], BF16, tag="sbf", name=f"sbf{r}")
            nc.vector.tensor_copy(sbf[:], seg_ps[r][:])
            s8 = segp.tile([P, 2, P], FP8, tag="s8", name=f"s8_{r}")
            nc.scalar.copy(s8[:], seg_ps[r][:])
            s8b = segp.tile([P, 2, P], BF16, tag="s8b", name=f"s8b{r}")
            nc.scalar.copy(s8b[:], s8[:])
            rbf = segp.tile([P, 2, P], BF16, tag="rbf", name=f"rbf{r}")
            nc.vector.tensor_tensor(
                rbf[:], sbf[:], s8b[:], op=mybir.AluOpType.subtract
            )
            r8 = segp.tile([P, 2, P], FP8, tag="r8", name=f"r8_{r}")
            nc.scalar.copy(r8[:], rbf[:])
            seg_t[r] = (s8, r8)

        ch_of = {}
        W2 = 256  # DoubleRow strip width (rhs free = 2*W2 = PE moving max)

        def emit_s2(r, p_lo=0, p_hi=8):
            # stage-2: per 256-strip, two fp8 DoubleRow matmuls (seg8 then
            # res8) contract all 256 groups at 0.5 cyc/row. Strips pair up in
            # one psum tile so each po->channel copy moves 512 columns.
            if r in ch_of:
                ch = ch_of[r]
            else:
                ch = chp.tile([P, N], BF16, tag="ch", name=f"ch{r}")
                ch_of[r] = ch
            s8, r8 = seg_t[r]
            for p_ in range(p_lo, p_hi):
                po = ps_o.tile([P, 2, W2], F32, tag="po", name=f"po{r}_{p_}")
                for half in range(2):
                    s = 2 * p_ + half
                    for op8 in (s8, r8):
                        nc.tensor.matmul(
                            po[:, half, :],
                            op8[:],
                            m2_sb[:, r, :, ts(s, W2)],
                            start=(op8 is s8),
                            stop=(op8 is r8),
                            perf_mode=mybir.MatmulPerfMode.DoubleRow,
                        )
                dve = p_ % 2 == 1 if r >= R - 2 else p_ % 4 == 2
                if dve:
                    nc.vector.tensor_copy(ch[:, ts(p_, 2 * W2)], po[:])
                else:
                    nc.scalar.copy(ch[:, ts(p_, 2 * W2)], po[:])
                # stream in quarters (2 pairs each): short final drain,
                # cheap per-DMA HWDGE hold (625ns, exclusive)
                if p_ % 2 == 1:
                    q = p_ // 2
                    nc.sync.dma_start(
                        out2[:, r, ts(q, N // 4)], ch[:, ts(q, N // 4)]
                    )

        # ---- software-pipelined emission: PE order is
        # s1(0) s1(1) s2(0) s1(2) s2(1) ... s1(5) s2(4) s2(5), so PE always
        # has a stage-1 queued while stage-2 deps (m2, seg copies) settle.
        # m2(r+1) is emitted during s2(r)'s window so DVE never idles on a
        # label DMA and each m2 is ready one full window early. ----
        masks0 = emit_m1(0)
        emit_s1(0, masks0)
        emit_seg_copy(0)
        masks1 = emit_m1(1)
        emit_s1(1, masks1)
        emit_seg_copy(1)
        for r in range(4):
            emit_s2(r)
            masks = emit_m1(r + 2)
            emit_s1(r + 2, masks)
            emit_seg_copy(r + 2)
        # tail: interleave ranks 4/5 stage-2 halves so rank 5's seg prep
        # settles during rank 4's strips and the final drain is ~0.5MB
        emit_s2(4, 0, 6)
        emit_s2(5, 0, 6)
        emit_s2(4, 6, 8)
        emit_s2(5, 6, 8)

    _legalize_waits(nc)
    return nc


def _host_tables(labels):
    # tabs16 = [iota_g | iota_p, iota_p+128 | labT] as u16
    iota_g = np.tile(np.arange(G, dtype=np.uint16), (P, 1))
    iota_p = (
        np.arange(P, dtype=np.uint16)[:, None]
        + np.uint16(128) * np.arange(2, dtype=np.uint16)[None, :]
    )
    labT = (
        np.transpose(labels.reshape(R, NT, P), (2, 0, 1))
        .reshape(P, R * NT)
        .astype(np.uint16)
    )
    return np.ascontiguousarray(
        np.concatenate([iota_g, iota_p, labT], axis=1).astype(np.uint16)
    )


def kernel(F_genus: np.ndarray, labels: np.ndarray) -> np.ndarray:
    F_genus = np.ascontiguousarray(F_genus, dtype=np.float32)
    labels = np.ascontiguousarray(labels, dtype=np.int32)
    assert F_genus.shape == (B, N) and labels.shape == (R, N)

    tabs16 = _host_tables(labels)
    # m28[r, g, h, j] = (labels[r, j] == h*128 + g) in fp8e4 (exact 0/1)
    gvals = (
        np.arange(P, dtype=np.int32)[None, :, None, None]
        + 128 * np.arange(2, dtype=np.int32)[None, None, :, None]
    )
    m28 = np.ascontiguousarray(
        (labels[:, None, None, :] == gvals).astype(ml_dtypes.float8_e4m3fn)
    )

    in_maps = []
    for c in range(NCORES):
        Fc = F_genus[c * BL : (c + 1) * BL]  # [BL, N]
        # f_t[p, t*128 + b] = Fc[b, t*128 + p]
        f_t = np.ascontiguousarray(
            Fc.reshape(BL, NT, P).transpose(2, 1, 0).reshape(P, N)
        ).astype(ml_dtypes.bfloat16)
        in_maps.append(
            {
                "f_t_in": f_t,
                "tabs16": tabs16,
                "m28": m28,
            }
        )

    # The first execution of a freshly compiled NEFF occasionally hits a
    # transient NRT_EXEC_UNIT_UNRECOVERABLE; a rebuild + retry recovers.
    last_err = None
    for attempt in range(3):
        try:
            if "nc" not in _cache:
                _cache["nc"] = _build_nc()
            res = run_bass_kernel_spmd(
                _cache["nc"], in_maps, core_ids=list(range(NCORES))
            )
            out = np.empty((B, N, R + 1), dtype=np.float32)
            for c in range(NCORES):
                # out2 is [BL, R, N] bf16, channel-major -> transpose
                out[c * BL : (c + 1) * BL, :, :R] = (
                    res.results[c]["out2"].astype(np.float32).transpose(0, 2, 1)
                )
            out[:, :, R] = F_genus  # identity channel, exact
            return out
        except Exception as e:  # noqa: BLE001
            last_err = e
            _cache.pop("nc", None)
            import time as _time

            _time.sleep(3.0)
    raise last_err
